# revision 2
# baseline (speedup 1.0000x reference)
"""DropSphereNd Trainium2 kernel.

Full computation (per sample n, channels c):
    activ = embeds @ table                      # [n, c]
    t     = 17th-smallest(activ, axis=1)        # [n, 1]
    out   = x * (activ >= t) * c/(c-16)

Sharding: data-parallel over batch n across 8 cores (x/embeds sharded,
table replicated).  Per core: x shard [8, 256, 56, 56] viewed as
[2048, 3136] = 16 tiles of [128, 3136]; tile t covers sample t//2,
channels (t%2)*128 + partition.

The whole kernel is DMA-fabric bound (16 SDMA engines x ~27 GB/s).  To
cut SBUF-side fabric bytes, x is staged in SBUF as bf16 via SWDGE
casting loads (gpsimd DMA, f32 HBM -> bf16 SBUF); the mask multiply
up-converts back to f32 tiles which HWDGE stores to HBM.  bf16
round-off (~4e-3 rel) is far inside the 2e-2 gate.

Raw bass (no Tile): all cross-engine deps use standalone wait_ge
sequencer commands; DMA sems tick in units of 16 (one per SDMA engine).

Engine plan:
  SP  (nc.sync)   - small input DMAs (table, embT, ident)
  POOL(nc.gpsimd) - 16 casting x-tile loads (SWDGE, f32->bf16)
  ACT (nc.scalar) - 16 f32 tile stores (HWDGE)
  PE  (nc.tensor) - projection matmul + 2 mask-transpose matmuls
  DVE (nc.vector) - threshold search, mask build, bf16->f32 scaled copies
"""

import sys

if "/opt/trn_rl_repo" not in sys.path:
    sys.path.insert(0, "/opt/trn_rl_repo")

from contextlib import ExitStack

import numpy as np

import concourse.bass as bass
from concourse import mybir
from concourse.bass_utils import run_bass_kernel_spmd

N, C, H, W = 64, 256, 56, 56
HW = H * W  # 3136
E = 16
NCORES = 8
NLOC = N // NCORES  # 8 samples per core
INDEX = 16  # ceil(C ** 0.5)
SCALE = float(C) / (C - INDEX)
F32 = mybir.dt.float32
BF16 = mybir.dt.bfloat16
NT = 2 * NLOC  # 16 tiles of [128, HW]
OBUFS = 8  # f32 out-tile ring slots

_NC_CACHE = {}


def _build_nc() -> bass.Bass:
    # detect_race_conditions only affects the interpreter: its raw-bass model
    # has no same-engine program-order edges, so every chained DVE op would be
    # flagged.  Cross-engine ordering is handled by the explicit sems below.
    nc = bass.Bass(detect_race_conditions=False)
    x = nc.dram_tensor("x", [NLOC * C, HW], F32, kind="ExternalInput")
    emb = nc.dram_tensor("embeds", [NLOC, E], F32, kind="ExternalInput")
    tab = nc.dram_tensor("table", [E, C], F32, kind="ExternalInput")
    out = nc.dram_tensor("out", [NLOC * C, HW], F32, kind="ExternalOutput")
    ident_d = nc.inline_tensor(np.eye(NLOC, dtype=np.float32), name="ident8")

    # tile t = rows t*128..(t+1)*128: sample t//2, channel (t%2)*128 + p
    x_t = x[:, :].rearrange("(t p) f -> t p f", p=128)
    o_t = out[:, :].rearrange("(t p) f -> t p f", p=128)

    with ExitStack() as ctx:
        sb = lambda name, shape, dt=F32: ctx.enter_context(
            nc.sbuf_tensor(name, shape, dt)
        )
        ps = lambda name, shape: ctx.enter_context(nc.psum_tensor(name, shape, F32))

        tab_s = sb("tab_s", [E, C])
        embT = sb("embT", [E, NLOC])
        ident = sb("ident", [NLOC, NLOC])
        v = sb("v", [NLOC, C])
        v2 = sb("v2", [NLOC, C])
        mx = sb("mx", [NLOC, 8])
        m_lo = sb("m_lo", [NLOC, C // 2])  # mask, channels 0-127
        m_hi = sb("m_hi", [NLOC, C // 2])  # mask, channels 128-255
        mA = sb("mA", [C // 2, NLOC])  # mask^T, channels 0-127
        mB = sb("mB", [C // 2, NLOC])  # mask^T, channels 128-255
        xbuf = [sb(f"xbuf{t}", [128, HW], BF16) for t in range(NT)]
        obuf = [sb(f"obuf{s}", [128, HW]) for s in range(OBUFS)]

        activ_p = ps("activ_p", [NLOC, C])
        mA_p = ps("mA_p", [C // 2, NLOC])
        mB_p = ps("mB_p", [C // 2, NLOC])

        ld = ctx.enter_context(nc.semaphore("ld"))
        fz = ctx.enter_context(nc.semaphore("fz"))
        dv = ctx.enter_context(nc.semaphore("dv"))
        pe = ctx.enter_context(nc.semaphore("pe"))
        xs = [ctx.enter_context(nc.semaphore(f"xs{t}")) for t in range(NT)]
        ss = [ctx.enter_context(nc.semaphore(f"ss{s}")) for s in range(OBUFS)]

        block = ctx.enter_context(nc.Block())

        # Smalls ride the SP HWDGE ring (the transposed embeds load alone is
        # 128 single-element descriptors); x tiles get the SWDGE ring.
        @block.sync
        def _(sync):
            sync.dma_start(out=tab_s[:, :], in_=tab[:, :]).then_inc(ld, 16)
            with nc.allow_non_contiguous_dma(reason="8x16 transposed load, 512B"):
                sync.dma_start(
                    out=embT[:, :], in_=emb[:, :].rearrange("n e -> e n")
                ).then_inc(ld, 16)
            sync.dma_start(out=ident[:, :], in_=ident_d[:, :]).then_inc(ld, 16)

        # SWDGE casting loads: f32 HBM -> bf16 SBUF, whole shard resident.
        @block.gpsimd
        def _(gpsimd):
            for t in range(NT):
                gpsimd.dma_start(out=xbuf[t][:, :], in_=x_t[t]).then_inc(xs[t], 16)

        @block.tensor
        def _(tensor):
            tensor.wait_ge(ld, 48)  # tab_s + embT (+ident) resident
            tensor.matmul(
                activ_p[:, :], embT[:, :], tab_s[:, :], start=True, stop=True
            ).then_inc(pe, 1)
            tensor.wait_ge(dv, 2)  # m_lo + m_hi built
            tensor.matmul(
                mA_p[:, :], m_lo[:, :], ident[:, :], start=True, stop=True
            ).then_inc(pe, 1)
            tensor.matmul(
                mB_p[:, :], m_hi[:, :], ident[:, :], start=True, stop=True
            ).then_inc(pe, 1)

        # The 16 smallest of activ == the 16 largest of v = -activ.  DVE's
        # max (top-8 per partition) + match_replace (zap those 8) drop them
        # in two rounds; surviving lanes keep their value, zapped lanes hold
        # MINV, so the mask is one compare against an immediate.  No
        # data-dependent scalar operands anywhere: TensorScalarPtr fetches
        # its scalar at sequencer dispatch (ahead of the DVE pipe), so only
        # mA/mB -- real pointer operands of the streaming muls -- need a
        # sem fence.
        MINV = -1.0e30

        @block.vector
        def _(vector):
            vector.wait_ge(pe, 1)
            vector.tensor_scalar_mul(v[:, :], activ_p[:, :], -1.0)
            # match_replace prefetches its 8-value table at dispatch, ahead
            # of the DVE pipe -- fence each max before consuming it
            vector.max(mx[:, :], v[:, :]).then_inc(fz, 1)
            vector.wait_ge(fz, 1)
            vector.match_replace(
                out=v2[:, :], in_to_replace=mx[:, :], in_values=v[:, :],
                imm_value=MINV,
            )
            vector.max(mx[:, :], v2[:, :]).then_inc(fz, 1)
            vector.wait_ge(fz, 2)
            vector.match_replace(
                out=v2[:, :], in_to_replace=mx[:, :], in_values=v2[:, :],
                imm_value=MINV,
            )
            # keep[c] <=> v2[c] != MINV ; mask = keep * SCALE, split into
            # low/high channel halves (immediate compare: real values are
            # > MINV/2)
            for lo, m8 in ((0, m_lo), (C // 2, m_hi)):
                vector.tensor_scalar(
                    out=m8[:, :],
                    in0=v2[:, lo : lo + C // 2],
                    scalar1=MINV / 2,
                    scalar2=SCALE,
                    op0=mybir.AluOpType.is_ge,
                    op1=mybir.AluOpType.mult,
                ).then_inc(dv, 1)
            vector.wait_ge(pe, 3)
            vector.tensor_copy(mA[:, :], mA_p[:, :])
            vector.tensor_copy(mB[:, :], mB_p[:, :]).then_inc(dv, 1)
            vector.wait_ge(dv, 3)  # mA/mB committed before mul ptr-fetches
            for t in range(NT):
                vector.wait_ge(xs[t], 16)
                if t >= OBUFS:
                    vector.wait_ge(ss[t % OBUFS], 16 * (t // OBUFS))
                m = mA if t % 2 == 0 else mB
                s = t // 2
                vector.tensor_scalar_mul(
                    obuf[t % OBUFS][:, :], xbuf[t][:, :], m[:, s : s + 1]
                ).then_inc(dv, 1)

        DV_BASE = 3  # dv value once masks + mA/mB copies are done

        @block.scalar
        def _(scalar):
            for t in range(NT):
                scalar.wait_ge(dv, DV_BASE + (t + 1))  # mul of tile t done
                scalar.dma_start(out=o_t[t], in_=obuf[t % OBUFS][:, :]).then_inc(
                    ss[t % OBUFS], 16
                )

    return nc


def _get_nc() -> bass.Bass:
    if "nc" not in _NC_CACHE:
        _NC_CACHE["nc"] = _build_nc()
    return _NC_CACHE["nc"]


def _in_maps(x, embeds, table):
    x = np.ascontiguousarray(np.asarray(x, dtype=np.float32))
    embeds = np.ascontiguousarray(np.asarray(embeds, dtype=np.float32))
    table = np.ascontiguousarray(np.asarray(table, dtype=np.float32))
    maps = []
    for i in range(NCORES):
        maps.append(
            {
                "x": x[i * NLOC : (i + 1) * NLOC].reshape(NLOC * C, HW),
                "embeds": embeds[i * NLOC : (i + 1) * NLOC],
                "table": table,
            }
        )
    return maps


def kernel(x, embeds, table):
    nc = _get_nc()
    res = run_bass_kernel_spmd(nc, _in_maps(x, embeds, table), list(range(NCORES)))
    shards = [
        np.asarray(res.results[i]["out"]).reshape(NLOC, C, H, W)
        for i in range(NCORES)
    ]
    return np.concatenate(shards, axis=0)


def kernel_profiled(x, embeds, table, **trace_kwargs):
    """Same as kernel() but with NTFF tracing; returns (output, BassKernelResults)."""
    nc = _get_nc()
    res = run_bass_kernel_spmd(
        nc, _in_maps(x, embeds, table), list(range(NCORES)), trace=True, **trace_kwargs
    )
    shards = [
        np.asarray(res.results[i]["out"]).reshape(NLOC, C, H, W)
        for i in range(NCORES)
    ]
    return np.concatenate(shards, axis=0), res
